# revision 28
# baseline (speedup 1.0000x reference)
"""Causal self-attention (B=4, T=2048, D=1024, H=16) on 8 trn2 NeuronCores.

Sharding: tensor-parallel over heads — 2 heads per core. Each core computes
qkv projections for its 2 heads (from replicated x), causal attention, and a
partial output projection (its 128 rows of w_proj). Host sums the 8 partial
[S, D] outputs.

Per-core kernel formulation (everything bf16 into the PE, fp32 PSUM accum):
  xT [D, S] (host-pretransposed)  ->  qT, kT = w.T @ xT  [128, S]
  vT = wv.T @ xT, then PE-transposed into v blocks [S, 128]
  scores (transposed): sT[j, i] = kT-as-lhsT @ qT-as-rhs, per (b, head),
    diagonal j-tiles sliced to [off:512] (below-diagonal cols skipped)
  p = exp(sT / 8) (ScalarE, causal-sliced; straddle tiles fused into one
    activate when the inter-head hole is small — its cols are never read),
    straddle diagonal masked by a lower-tri multiply split vector/gpsimd
  oT[d, i] = [v_h | ones].T @ p accumulated over j tiles; the ones columns
    produce the softmax denominator rows for free
  normalize per chunk: both heads' denominator rows staged into one
    partition-0/32 tile, a single fused reciprocal_approx_fast, bf16
    casts, K=1 broadcast matmuls (e0/e1), then a tensor_mul over oT
  out partial = oT-as-lhsT @ w_proj-rows-as-rhs  [S, D] -> bf16 -> HBM,
    each 512-col half DMA'd as soon as it is staged

Scheduling: the scalar engine's exp rate (~260ns fixed + ~0.83ns/col) is
slightly slower than the PE's scores+AV work per j-tile, so a filler queue
of deferred PE units (v-transposes + the previous chunk's norm/proj) is
woven into the attention jt loop (two units per flush when backlogged,
except through batch 2 — the backlog spills into batch 3, whose attention
has no qkv blocks left to fill with), and AV matmuls trail their exp by
three jts. qkv for batch b+1 runs as a pure-PE block after each attention
chunk; its qT/kT/vT psum->sbuf copies run on the scalar engine inside
those windows (scalar idles there anyway), its x tiles are DMA-prefetched
one attention chunk ahead as two 3D descriptors, and chunk 0's x rides
between the split weight DMAs at startup. Steady-state transpose units
append to the queue (a batch of slack) so their pjbc-psum allocations
stay clear of the ps_v/pj ring positions; batch 0's jump the front.
"""

import math
import os
from collections import deque

import numpy as np
import ml_dtypes

# must be set before NRT initializes: recover cleanly if a previous
# process left a core wedged
os.environ.setdefault("NEURON_RT_RESET_CORES", "1")

B, T, D, H = 4, 2048, 1024, 16
HD = D // H           # 64
S = B * T             # 8192
P = 128
KT = D // P           # 8 k-tiles over D
MC = S // 512         # 16 m-chunks of 512
NT = S // P           # 64 m-tiles of 128
JT = T // P           # 16 j-tiles per batch
NCH = T // 512        # 4 i-chunks per batch
N_CORES = 8

BFNP = ml_dtypes.bfloat16

_CACHE = {}


def _build_nc():
    import concourse.tile as tile
    import concourse.mybir as mybir
    from concourse import bacc

    BF = mybir.dt.bfloat16
    F32 = mybir.dt.float32
    Exp = mybir.ActivationFunctionType.Exp

    nc = bacc.Bacc("TRN2", num_devices=N_CORES)

    xT = nc.dram_tensor("xT", [D, S], BF, kind="ExternalInput").ap()
    # weights pre-rearranged host-side to [p, kt*128+n] so the DMA is
    # a contiguous [128, 1024] transfer (2KB per partition)
    wq = nc.dram_tensor("wq", [P, KT * P], BF, kind="ExternalInput").ap()
    wk = nc.dram_tensor("wk", [P, KT * P], BF, kind="ExternalInput").ap()
    wv = nc.dram_tensor("wv", [P, KT * P], BF, kind="ExternalInput").ap()
    wp = nc.dram_tensor("wp", [P, D], BF, kind="ExternalInput").ap()
    maskt = nc.dram_tensor("maskt", [P, P], BF, kind="ExternalInput").ap()
    e01 = nc.dram_tensor("e01", [2, P], BF, kind="ExternalInput").ap()
    ident = nc.dram_tensor("ident", [P, P], BF, kind="ExternalInput").ap()
    out_p = nc.dram_tensor("out_p", [S, D], BF, kind="ExternalOutput").ap()

    with tile.TileContext(nc) as tc:
        with tc.tile_pool(name="singles", bufs=1) as singles:
            qT_sb = singles.tile([P, S], BF)
            kT_sb = singles.tile([P, S], BF)
            oT_sb = singles.tile([P, S], BF)
            # v blocks per m-tile: [v_h0 | ones | v_h1 | ones] (65-wide lhsTs)
            v_sb = singles.tile([P, NT, 130], BF)
            wq_sb = singles.tile([P, KT, P], BF)
            wk_sb = singles.tile([P, KT, P], BF)
            wv_sb = singles.tile([P, KT, P], BF)
            wp_sb = singles.tile([P, D], BF)
            mask_sb = singles.tile([P, P], BF)
            e0_sb = singles.tile([1, P], BF)
            e1_sb = singles.tile([1, P], BF)
            id_sb = singles.tile([P, P], BF)
            vT_sb = singles.tile([P, S], BF)

            wq_r = wq.rearrange("p (kt n) -> p kt n", n=P)
            wk_r = wk.rearrange("p (kt n) -> p kt n", n=P)
            wv_r = wv.rearrange("p (kt n) -> p kt n", n=P)
            xT_r = xT.rearrange("(kt p) m -> p kt m", p=P)

            nc.vector.memset(v_sb[:, :, 64:65], 1.0)
            nc.vector.memset(v_sb[:, :, 129:130], 1.0)

            # PSUM budget (8 banks): s_ps bufs=2 of [128,1024] (4 banks)
            # + av_0/av_1 bufs=1 (2) + shared pjbc tag bufs=2 (2).
            with (
                tc.tile_pool(name="xc_pool", bufs=5) as xpool,
                tc.tile_pool(name="p_pool", bufs=8) as ppool,
                tc.tile_pool(name="rf_pool", bufs=3) as rfp,
                tc.tile_pool(name="rb_pool", bufs=6) as rbp,
                tc.tile_pool(name="out_pool", bufs=8) as outp,
                tc.tile_pool(name="ps_s", bufs=2, space="PSUM") as ps2,
                tc.tile_pool(name="ps_av", bufs=1, space="PSUM") as avp,
                tc.tile_pool(name="ps_pj", bufs=2, space="PSUM") as ps3,
            ):
                filler_q = deque()  # entries: (is_ordered, fn)
                hold_backlog = [False]

                def pop_filler():
                    # hold the backlog while batch 2 runs: batch 3 has no
                    # qkv blocks, so spilled norm/proj units are its only
                    # PE fill during the scalar-bound attention sections
                    n = 2 if (len(filler_q) > 10 and not hold_backlog[0]) else 1
                    for _ in range(min(n, len(filler_q))):
                        filler_q.popleft()[1]()

                def transpose_unit(mt):
                    def run():
                        ps_t = ps3.tile([P, P], BF, name="ps_t", tag="pjbc")
                        nc.tensor.transpose(
                            ps_t, vT_sb[:, mt * P:(mt + 1) * P], id_sb)
                        nc.vector.tensor_copy(out=v_sb[:, mt, 0:64],
                                              in_=ps_t[:, 0:64])
                        nc.vector.tensor_copy(out=v_sb[:, mt, 65:129],
                                              in_=ps_t[:, 64:128])
                    return run

                def norm_unit(b, c, rb0, rb1):
                    def run():
                        bc_ps = ps3.tile([P, 512], F32, name="bc", tag="pjbc")
                        nc.tensor.matmul(bc_ps, lhsT=e0_sb, rhs=rb0,
                                         start=True, stop=False)
                        nc.tensor.matmul(bc_ps, lhsT=e1_sb, rhs=rb1,
                                         start=False, stop=True)
                        sl = slice(b * T + c * 512, b * T + (c + 1) * 512)
                        nc.vector.tensor_mul(out=oT_sb[:, sl], in0=oT_sb[:, sl],
                                             in1=bc_ps)
                    return run

                ob_map = {}

                def proj_unit(b, c, i, nch):
                    def run():
                        mt = (b * T + c * 512) // P + i
                        if nch == 0:
                            ob_map[mt] = outp.tile([P, D], BF, name="ob")
                        ob = ob_map[mt]
                        pj = ps3.tile([P, 512], F32, name="pj", tag="pjbc")
                        nc.tensor.matmul(
                            pj, lhsT=oT_sb[:, mt * P:(mt + 1) * P],
                            rhs=wp_sb[:, nch * 512:(nch + 1) * 512],
                            start=True, stop=True)
                        if b == B - 1 and c == NCH - 1 and nch == 0:
                            # post-exp tail: scalar is free — alternate the
                            # stage copies so the pj chain drains 2x faster
                            nc.scalar.copy(
                                out=ob[:, nch * 512:(nch + 1) * 512], in_=pj)
                        else:
                            nc.vector.tensor_copy(
                                out=ob[:, nch * 512:(nch + 1) * 512], in_=pj)
                        # DMA each half as soon as it is staged
                        nc.sync.dma_start(
                            out=out_p[mt * P:(mt + 1) * P,
                                      nch * 512:(nch + 1) * 512],
                            in_=ob[:, nch * 512:(nch + 1) * 512])
                    return run

                def attention_chunk(b, c):
                    av_t = [avp.tile([P, 512], F32, name=f"av_{h}")
                            for h in (0, 1)]
                    # software pipeline: AV three jts behind its exp, so the
                    # PE never waits on a just-issued activation
                    pending_av = deque()  # per-jt groups of matmul args
                    njt = 4 * c + 4

                    def flush_av(keep=3):
                        pop_filler()
                        while len(pending_av) > keep:
                            for args in pending_av.popleft():
                                nc.tensor.matmul(*args[0], **args[1])

                    for jt in range(njt):
                        diag = (jt // 4 == c)
                        off = jt * P - c * 512 if diag else 0
                        # both heads' scores in one 2-bank psum tile,
                        # diagonal tiles sliced to the causal span
                        s_ps = ps2.tile([P, 1024], F32, name="s_ps")
                        for h in (0, 1):
                            lk = kT_sb[h * 64:(h + 1) * 64,
                                       b * T + jt * P: b * T + (jt + 1) * P]
                            rq = qT_sb[h * 64:(h + 1) * 64,
                                       b * T + c * 512 + off: b * T + (c + 1) * 512]
                            nc.tensor.matmul(s_ps[:, 512 * h + off: 512 * (h + 1)],
                                             lhsT=lk, rhs=rq,
                                             start=True, stop=True)
                        flush_av()
                        p_sb = ppool.tile([P, 1024], BF, name="p_sb")
                        scale = 1.0 / math.sqrt(HD)
                        if off <= 256:
                            # single exp spanning both heads; for sliced
                            # diagonals the inter-head hole cols hold stale
                            # finite psum values whose exp is never read —
                            # one activate amortizes the fixed issue cost
                            nc.scalar.activation(
                                out=p_sb[:, off:1024], in_=s_ps[:, off:1024],
                                func=Exp, scale=scale)
                        else:
                            for h in (0, 1):
                                nc.scalar.activation(
                                    out=p_sb[:, 512 * h + off: 512 * (h + 1)],
                                    in_=s_ps[:, 512 * h + off: 512 * (h + 1)],
                                    func=Exp, scale=scale)
                        if diag:
                            # both masks on gpsimd: DVE is the loaded
                            # engine, and the 2-jt AV lag hides the latency
                            nc.gpsimd.tensor_mul(
                                out=p_sb[:, off: off + P],
                                in0=p_sb[:, off: off + P],
                                in1=mask_sb)
                            nc.gpsimd.tensor_mul(
                                out=p_sb[:, 512 + off: 512 + off + P],
                                in0=p_sb[:, 512 + off: 512 + off + P],
                                in1=mask_sb)
                        group = []
                        for h in (0, 1):
                            lv = v_sb[:, b * JT + jt, 65 * h: 65 * h + 65]
                            group.append((
                                (av_t[h][0:65, off:512],),
                                dict(lhsT=lv,
                                     rhs=p_sb[:, 512 * h + off: 512 * (h + 1)],
                                     start=(jt == 0), stop=(jt == 4 * c + 3)),
                            ))
                        pending_av.append(group)
                    flush_av(keep=0)
                    # chunk end: both heads' denominator rows (psum partition
                    # 64) staged into one partition-0/32 tile, one fused
                    # reciprocal, bf16 casts
                    # (NOTE: reciprocal_approx_fast misreads nonzero
                    # partition offsets on HW — hence the aligned stage)
                    sl = slice(b * T + c * 512, b * T + (c + 1) * 512)
                    stg = rfp.tile([33, 512], F32, name="stg")
                    # on the very last chunk scalar is free (no more exps):
                    # run the stage copies there so the reciprocal->norm->
                    # proj tail chain starts sooner
                    ceng = nc.scalar if (b == B - 1 and c == NCH - 1) else None
                    for h in (0, 1):
                        if ceng is not None:
                            ceng.copy(out=stg[32 * h:32 * h + 1, :],
                                      in_=av_t[h][64:65, :])
                            ceng.copy(out=oT_sb[h * 64:(h + 1) * 64, sl],
                                      in_=av_t[h][0:64, :])
                        else:
                            nc.vector.tensor_copy(
                                out=stg[32 * h:32 * h + 1, :],
                                in_=av_t[h][64:65, :])
                            nc.vector.tensor_copy(
                                out=oT_sb[h * 64:(h + 1) * 64, sl],
                                in_=av_t[h][0:64, :])
                    rst = rfp.tile([33, 512], F32, name="rst")
                    nc.vector.reciprocal_approx_fast(out=rst, in_=stg)
                    rbs = []
                    for h in (0, 1):
                        rb = rbp.tile([1, 512], BF, name=f"rb{h}")
                        nc.vector.tensor_copy(out=rb,
                                              in_=rst[32 * h:32 * h + 1, :])
                        rbs.append(rb)
                    filler_q.append((False, norm_unit(b, c, rbs[0], rbs[1])))
                    for i in range(4):
                        for nch in range(2):
                            filler_q.append((False, proj_unit(b, c, i, nch)))

                def dma_xc(mc):
                    sl = slice(mc * 512, (mc + 1) * 512)
                    xc = xpool.tile([P, KT, 512], BF, name="xc")
                    # two 3D descriptors for the chunk (two queues): the
                    # sync engine's ~600ns per-descriptor cost is scarce,
                    # and the transfer hides behind a one-chunk prefetch
                    nc.sync.dma_start(out=xc[:, 0:4], in_=xT_r[:, 0:4, sl])
                    nc.sync.dma_start(out=xc[:, 4:KT], in_=xT_r[:, 4:KT, sl])
                    return xc

                def qkv_chunk(mc, xc):
                    sl = slice(mc * 512, (mc + 1) * 512)
                    s_qk = ps2.tile([P, 1024], F32, name="s_qk", tag="s_ps")
                    for kt in range(KT):
                        nc.tensor.matmul(s_qk[:, 0:512], lhsT=wq_sb[:, kt],
                                         rhs=xc[:, kt],
                                         start=(kt == 0), stop=(kt == KT - 1))
                        nc.tensor.matmul(s_qk[:, 512:1024], lhsT=wk_sb[:, kt],
                                         rhs=xc[:, kt],
                                         start=(kt == 0), stop=(kt == KT - 1))
                    # psum->sbuf copies on the scalar engine: it idles in
                    # these qkv windows (nothing to exp), keeping DVE free
                    nc.scalar.copy(out=qT_sb[:, sl], in_=s_qk[:, 0:512])
                    nc.scalar.copy(out=kT_sb[:, sl], in_=s_qk[:, 512:1024])
                    ps_v = ps3.tile([P, 512], F32, name="ps_v", tag="pjbc")
                    for kt in range(KT):
                        nc.tensor.matmul(ps_v, lhsT=wv_sb[:, kt], rhs=xc[:, kt],
                                         start=(kt == 0), stop=(kt == KT - 1))
                    nc.scalar.copy(out=vT_sb[:, sl], in_=ps_v)
                    return [(True, transpose_unit(mc * 4 + i)) for i in range(4)]

                # qkv weights first — they gate the very first matmuls;
                # split so the first s_qk matmuls unblock after a quarter
                # of the weight bytes have landed
                nc.sync.dma_start(out=wq_sb[:, 0:2], in_=wq_r[:, 0:2])
                nc.sync.dma_start(out=wk_sb[:, 0:2], in_=wk_r[:, 0:2])
                # chunk 0's first two x tiles land right behind the small
                # weight slices: the first matmuls unblock on ~400KB while
                # the bulk weight bytes stream in behind
                xc0 = xpool.tile([P, KT, 512], BF, name="xc")
                for kt in range(2):
                    nc.sync.dma_start(out=xc0[:, kt], in_=xT_r[:, kt, 0:512])
                nc.sync.dma_start(out=wq_sb[:, 2:KT], in_=wq_r[:, 2:KT])
                nc.sync.dma_start(out=wk_sb[:, 2:KT], in_=wk_r[:, 2:KT])
                for kt in range(2, KT):
                    nc.sync.dma_start(out=xc0[:, kt], in_=xT_r[:, kt, 0:512])
                nc.sync.dma_start(out=wv_sb, in_=wv_r)

                def late_const_dmas():
                    nc.sync.dma_start(out=id_sb, in_=ident)
                    nc.sync.dma_start(out=wp_sb, in_=wp)
                    nc.sync.dma_start(out=mask_sb, in_=maskt)
                    nc.sync.dma_start(out=e0_sb, in_=e01[0:1, :])
                    nc.sync.dma_start(out=e1_sb, in_=e01[1:2, :])

                # batch pipeline: attention(b) weaves transpose + norm/proj
                # filler units; qkv(b+1, c) follows each attention chunk as
                # a pure-PE block. batch 0's transpose units jump the queue
                # (its AV matmuls need v_sb blocks right away); steady-state
                # transposes append — they are only needed a batch later,
                # and spacing them from the ps_v/pj allocations avoids pjbc
                # ring collisions
                tunits = qkv_chunk(0, xc0)
                late_const_dmas()
                for mc in range(1, 4):
                    tunits += qkv_chunk(mc, dma_xc(mc))
                filler_q.extendleft(reversed(tunits))
                xc_next = dma_xc(4)
                for b in range(B):
                    hold_backlog[0] = (b == 2)
                    for c in range(NCH):
                        attention_chunk(b, c)
                        if b + 1 < B:
                            xc_cur = xc_next
                            mc = 4 * (b + 1) + c
                            if mc + 1 < MC:
                                xc_next = dma_xc(mc + 1)
                            filler_q.extend(qkv_chunk(mc, xc_cur))
                while filler_q:
                    filler_q.popleft()[1]()

    nc.compile()
    return nc


def _host_inputs(x, w_qkv, w_proj):
    x = np.asarray(x, dtype=np.float32)
    w_qkv = np.asarray(w_qkv, dtype=np.float32)
    w_proj = np.asarray(w_proj, dtype=np.float32)

    xT = np.ascontiguousarray(x.reshape(S, D).T).astype(BFNP)
    mask = np.triu(np.ones((P, P), np.float32)).astype(BFNP)  # [j, i]: 1 if j<=i
    e01 = np.zeros((2, P), np.float32)
    e01[0, :64] = 1.0
    e01[1, 64:] = 1.0
    e01 = e01.astype(BFNP)
    ident = np.eye(P, dtype=np.float32).astype(BFNP)

    def wslice(w, c0):
        # [D, 128] -> [p, kt*128+n] so the device DMA is contiguous
        blk = w[:, c0:c0 + P].reshape(KT, P, P).transpose(1, 0, 2)
        return np.ascontiguousarray(blk.reshape(P, KT * P)).astype(BFNP)

    in_maps = []
    for core in range(N_CORES):
        cs = slice(core * P, (core + 1) * P)
        in_maps.append({
            "xT": xT,
            "wq": wslice(w_qkv, core * P),
            "wk": wslice(w_qkv, D + core * P),
            "wv": wslice(w_qkv, 2 * D + core * P),
            "wp": np.ascontiguousarray(w_proj[cs, :]).astype(BFNP),
            "maskt": mask,
            "e01": e01,
            "ident": ident,
        })
    return in_maps


def run_spmd(x, w_qkv, w_proj, trace=False):
    """Compile (cached) + run on 8 cores. Returns (out [B,T,D] fp32, results)."""
    from concourse import bass_utils

    if "nc" not in _CACHE:
        _CACHE["nc"] = _build_nc()
    nc = _CACHE["nc"]

    in_maps = _host_inputs(x, w_qkv, w_proj)
    res = bass_utils.run_bass_kernel_spmd(
        nc, in_maps, core_ids=list(range(N_CORES)), trace=trace)

    acc = np.zeros((S, D), np.float32)
    for r in res.results:
        acc += np.asarray(r["out_p"]).astype(np.float32)
    return acc.reshape(B, T, D), res


def kernel(x, w_qkv, w_proj):
    try:
        out, _ = run_spmd(x, w_qkv, w_proj, trace=False)
    except Exception:
        # rare transient device wedge — one retry on the cached NEFF
        out, _ = run_spmd(x, w_qkv, w_proj, trace=False)
    return out
